# revision 3
# baseline (speedup 1.0000x reference)
"""BasicTransDecoderBlock on 8 Trainium2 NeuronCores — full on-device pipeline.

Architecture (one warm call = 2 async H2D puts + 3 chained async execs +
1 blocking fetch, ~1 tunnel RTT total):

  1. x1 (256,512) and x2 (128,4096) ship sharded over channels (2.6MB).
  2. pre-glue shard_map jit: all-gather x1/x2, then each core redundantly
     computes BN + depthwise/pointwise convs + align-corners interp
     (as einsums with a constant (16,8) matrix) + residue path, and emits
     its per-core bass inputs: kT/qT/VA (bf16) + donated OT zeros.
  3. bass attention exec (unchanged math from the working baseline):
     4 heads x 4096 queries x 4096 keys, query-sharded 512/core.
     exp(s(qk+bias)) = exp(s qk) * WE window table (rel-pos bias is
     affine in u=31h+w+d; each core's queries span 62 consecutive u),
     softmax normalization folded into AV via an appended ones column.
  4. post-glue shard_map jit: all-gather OT, normalize, reassemble
     channels, out conv + residual + BN/relu + mlp conv + residual,
     replicated output.
  5. single np.asarray fetch of the replicated (1,128,16,16,16) result.

Weights (incl. the host-built 16MB WE exp-bias table, a pure function of
rel_table) are cached device-resident keyed on their bytes.
"""

import sys
import numpy as np

sys.path.insert(0, "/opt/trn_rl_repo")

import ml_dtypes

IN_CH, OUT_CH, HEADS, DIM_HEAD, R = 256, 128, 4, 32, 16
EPS = 1e-5
SCALE = DIM_HEAD ** -0.5
N = R * R * R          # 4096 keys / queries
QPC = N // 8           # 512 queries per core
CWIN = 62              # distinct u values per core (2 h-planes)
CPAD = 64              # padded bias window stride
VPAD = 36              # 32 dims + 1 ones col, padded
NCORES = 8

WNAMES = ["w_ch", "b_ch", "gamma_l", "beta_l", "gamma_h", "beta_h",
          "gamma2", "beta2", "kv_dw", "kv_pw", "q_dw", "q_pw",
          "out_dw", "out_pw", "w_mlp"]


# ---------------- host-side numpy reference pieces (fallback) ----------------

def _pw(x, w):
    b, c = x.shape[0], x.shape[1]
    xf = x.reshape(b, c, -1)
    o = np.einsum("oi,bif->bof", w.reshape(w.shape[0], c), xf)
    return o.reshape(b, w.shape[0], *x.shape[2:])


def _dw(x, wd):
    b, c, h, w, d = x.shape
    xp = np.zeros((b, c, h + 2, w + 2, d + 2), x.dtype)
    xp[:, :, 1:-1, 1:-1, 1:-1] = x
    out = np.zeros_like(x)
    for a in range(3):
        for bb in range(3):
            for cc in range(3):
                out += wd[None, :, 0, a, bb, cc, None, None, None] * \
                    xp[:, :, a:a + h, bb:bb + w, cc:cc + d]
    return out


def _bn(x, g, b):
    m = x.mean(axis=(0, 2, 3, 4), keepdims=True, dtype=np.float32)
    v = ((x - m) ** 2).mean(axis=(0, 2, 3, 4), keepdims=True, dtype=np.float32)
    return (x - m) / np.sqrt(v + EPS) * g.reshape(1, -1, 1, 1, 1) + \
        b.reshape(1, -1, 1, 1, 1)


def _interp_mat(out_len, in_len):
    I = np.zeros((out_len, in_len), np.float32)
    pos = np.arange(out_len, dtype=np.float64) * ((in_len - 1) / (out_len - 1))
    lo = np.clip(np.floor(pos).astype(np.int64), 0, in_len - 1)
    hi = np.clip(lo + 1, 0, in_len - 1)
    w = (pos - lo).astype(np.float32)
    for o in range(out_len):
        I[o, lo[o]] += 1.0 - w[o]
        I[o, hi[o]] += w[o]
    return I


def _interp1(x, axis, out_len):
    in_len = x.shape[axis]
    if in_len == out_len:
        return x
    I = _interp_mat(out_len, in_len)
    return np.moveaxis(np.tensordot(I, np.moveaxis(x, axis, 0), axes=1), 0, axis)


def _interp3(x, size):
    for ax, s in zip((2, 3, 4), size):
        x = _interp1(x, ax, s)
    return x


def _u_vec():
    hh, ww, dd = np.meshgrid(np.arange(R), np.arange(R), np.arange(R),
                             indexing="ij")
    return (31 * hh + ww + dd).reshape(-1)  # (4096,), u in [0,495]


def _t_ext(rel_table):
    # t_ext[h, m] for m in [-15, 975] stored at index m+15 -> length 991
    m = np.arange(-15, 976) % ((2 * R - 1) ** 3)
    return rel_table[m, :].T.astype(np.float32)  # (4, 991)


# ---------------- device kernel ----------------

_CACHE = {}


def _ap4(t, ap_dims):
    import concourse.bass as bass
    b = t if isinstance(t, bass.AP) else t[:]
    return bass.AP(tensor=b.tensor, offset=b.offset,
                   ap=[list(b.ap[0])] + ap_dims)


def _build_bass():
    import concourse.bass as bass
    import concourse.mybir as mybir
    from contextlib import ExitStack

    dt = mybir.dt
    nc = bass.Bass()
    kT = nc.dram_tensor("kT", [HEADS, 32, N], dt.bfloat16, kind="ExternalInput")
    qT = nc.dram_tensor("qT", [HEADS, 32, QPC], dt.bfloat16, kind="ExternalInput")
    VA = nc.dram_tensor("VA", [128, HEADS * 32 * VPAD], dt.bfloat16,
                        kind="ExternalInput")
    WE = nc.dram_tensor("WE", [128, HEADS * 32 * CPAD], dt.bfloat16,
                        kind="ExternalInput")
    OT = nc.dram_tensor("OT", [HEADS, VPAD, QPC], dt.float32,
                        kind="ExternalOutput")

    T = HEADS * 32  # 128 pipeline steps
    NB = 2          # double buffering

    with ExitStack() as ctx:
        en = ctx.enter_context
        kT_sb = en(nc.sbuf_tensor("kT_sb", [32, HEADS * N], dt.bfloat16))
        qT_sb = en(nc.sbuf_tensor("qT_sb", [32, HEADS * QPC], dt.bfloat16))
        va_sb = en(nc.sbuf_tensor("va_sb", [128, HEADS * 32 * VPAD], dt.bfloat16))
        we_sb = en(nc.sbuf_tensor("we_sb", [128, HEADS * 32 * CPAD], dt.bfloat16))
        e_sb = [en(nc.sbuf_tensor(f"e_sb{i}", [128, QPC], dt.bfloat16)) for i in range(NB)]
        p_sb = [en(nc.sbuf_tensor(f"p_sb{i}", [128, QPC], dt.bfloat16)) for i in range(NB)]
        ob_sb = [en(nc.sbuf_tensor(f"ob_sb{i}", [VPAD, QPC], dt.float32)) for i in range(HEADS)]
        pq_ps = [en(nc.psum_tensor(f"pq_ps{i}", [128, QPC], dt.float32)) for i in range(NB)]
        po_ps = [en(nc.psum_tensor(f"po_ps{i}", [VPAD, QPC], dt.float32)) for i in range(HEADS)]

        dmas = en(nc.semaphore("dmas"))
        qks = en(nc.semaphore("qks"))
        acts = en(nc.semaphore("acts"))
        dvs = en(nc.semaphore("dvs"))
        avs = en(nc.semaphore("avs"))
        cps = en(nc.semaphore("cps"))
        blk = en(nc.Block())

        NDMA = 2 * HEADS + 2

        @blk.sync
        def _(s):
            for h in range(HEADS):
                s.dma_start(kT_sb[:, h * N:(h + 1) * N], kT[h]).then_inc(dmas, 16)
                s.dma_start(qT_sb[:, h * QPC:(h + 1) * QPC], qT[h]).then_inc(dmas, 16)
            s.dma_start(va_sb[:], VA[:]).then_inc(dmas, 16)
            s.dma_start(we_sb[:], WE[:]).then_inc(dmas, 16)
            for h in range(HEADS):
                s.wait_ge(cps, h + 1)
                s.dma_start(OT[h], ob_sb[h][:]).then_inc(dmas, 16)

        @blk.tensor
        def _(t):
            t.wait_ge(dmas, 16 * NDMA)
            for ti in range(T):
                h, jc = ti // 32, ti % 32
                if ti >= 2:
                    t.wait_ge(acts, ti - 1)
                t.matmul(pq_ps[ti % NB][:],
                         kT_sb[:, h * N + jc * 128: h * N + (jc + 1) * 128],
                         qT_sb[:, h * QPC:(h + 1) * QPC],
                         start=True, stop=True).then_inc(qks, 1)
                if ti >= 1:
                    tp = ti - 1
                    hp, jp = tp // 32, tp % 32
                    t.wait_ge(dvs, tp + 1)
                    t.matmul(po_ps[hp][:],
                             va_sb[:, tp * VPAD:(tp + 1) * VPAD],
                             p_sb[tp % NB][:],
                             start=(jp == 0), stop=(jp == 31)).then_inc(avs, 1)
            tp = T - 1
            t.wait_ge(dvs, tp + 1)
            t.matmul(po_ps[HEADS - 1][:],
                     va_sb[:, tp * VPAD:(tp + 1) * VPAD],
                     p_sb[tp % NB][:],
                     start=False, stop=True).then_inc(avs, 1)

        @blk.scalar
        def _(s):
            for ti in range(T):
                s.wait_ge(qks, ti + 1)
                if ti >= NB:
                    s.wait_ge(dvs, ti - 1)
                s.activation(e_sb[ti % NB][:], pq_ps[ti % NB][:],
                             mybir.ActivationFunctionType.Exp,
                             scale=float(SCALE)).then_inc(acts, 1)

        @blk.vector
        def _(v):
            v.wait_ge(dmas, 16 * NDMA)
            for ti in range(T):
                v.wait_ge(acts, ti + 1)
                if ti >= NB:
                    v.wait_ge(avs, ti - 1)
                base = we_sb[:, ti * CPAD: ti * CPAD + CWIN]
                w_ap = _ap4(base, [[31, 2], [1, 16], [1, 16]])
                e4 = _ap4(e_sb[ti % NB], [[256, 2], [16, 16], [1, 16]])
                p4 = _ap4(p_sb[ti % NB], [[256, 2], [16, 16], [1, 16]])
                v.tensor_tensor(p4, e4, w_ap,
                                op=mybir.AluOpType.mult).then_inc(dvs, 1)
            for h in range(HEADS):
                v.wait_ge(avs, 32 * (h + 1))
                v.tensor_copy(ob_sb[h][:], po_ps[h][:]).then_inc(cps, 1)
    return nc


def _build_we_concat(rel_table):
    """Per-core exp-bias window tables, concatenated core-major:
    (8*128, HEADS*32*CPAD) bf16."""
    bf16 = ml_dtypes.bfloat16
    u = _u_vec()
    te = _t_ext(rel_table)                                     # (4, 991)
    tabs = np.exp(SCALE * te)                                  # (4, 991)
    cc = np.arange(CWIN)
    base_midx = cc[None, :] + 495 - u[:, None]                 # (4096, 62)
    out = np.zeros((NCORES, 128, HEADS, 32, CPAD), np.float32)
    for c in range(NCORES):
        # WE[p, (h*32+jc)*CPAD + cc] = exp(SCALE * t_ext[h, cc + 62c - u_j + 480])
        # with j = jc*128 + p
        g = tabs[:, base_midx + 62 * c]                        # (4, 4096, 62)
        out[c, :, :, :, :CWIN] = g.reshape(HEADS, 32, 128, CWIN).transpose(2, 0, 1, 3)
    return np.ascontiguousarray(
        out.reshape(NCORES * 128, HEADS * 32 * CPAD)).astype(bf16)


def _get_ctx():
    """Build-once runner state: bass jit, pre/post glue jits, shardings."""
    if "ctx" in _CACHE:
        return _CACHE["ctx"]

    import jax
    import jax.numpy as jnp
    from jax.experimental.shard_map import shard_map
    from jax.sharding import Mesh, NamedSharding, PartitionSpec as P
    import concourse.mybir as mybir
    from concourse import bass2jax

    bass2jax.install_neuronx_cc_hook()
    nc = _build_bass()

    partition_name = (nc.partition_id_tensor.name
                      if nc.partition_id_tensor else None)
    in_names, out_names, out_avals = [], [], []
    for alloc in nc.m.functions[0].allocations:
        if not isinstance(alloc, mybir.MemoryLocationSet):
            continue
        name = alloc.memorylocations[0].name
        if alloc.kind == "ExternalInput":
            if name != partition_name:
                in_names.append(name)
        elif alloc.kind == "ExternalOutput":
            out_avals.append(jax.core.ShapedArray(
                tuple(alloc.tensor_shape), mybir.dt.np(alloc.dtype)))
            out_names.append(name)
    n_params, n_outs = len(in_names), len(out_names)
    all_names = list(in_names) + list(out_names)
    if partition_name is not None:
        all_names.append(partition_name)

    def _body(*args):
        operands = list(args)
        if partition_name is not None:
            operands.append(bass2jax.partition_id_tensor())
        outs = bass2jax._bass_exec_p.bind(
            *operands,
            out_avals=tuple(out_avals),
            in_names=tuple(all_names),
            out_names=tuple(out_names),
            lowering_input_output_aliases=(),
            sim_require_finite=True,
            sim_require_nnan=True,
            nc=nc,
        )
        return tuple(outs)

    devices = jax.devices()[:NCORES]
    assert len(devices) == NCORES, f"need {NCORES} cores, got {len(devices)}"
    mesh = Mesh(np.asarray(devices), ("core",))
    shard = NamedSharding(mesh, P("core"))
    repl = NamedSharding(mesh, P())
    in_specs = (P("core"),) * (n_params + n_outs)
    out_specs = (P("core"),) * n_outs
    donate = tuple(range(n_params, n_params + n_outs))
    sharded = jax.jit(
        shard_map(_body, mesh=mesh, in_specs=in_specs, out_specs=out_specs,
                  check_rep=False),
        donate_argnums=donate,
        keep_unused=True,
    )

    I16 = jnp.asarray(_interp_mat(R, 8))                       # (16, 8)
    bf = jnp.bfloat16

    def bn2(x, g, b):
        # x (C, S): batch stats over S
        m = jnp.mean(x, axis=1, keepdims=True)
        v = jnp.mean((x - m) * (x - m), axis=1, keepdims=True)
        return (x - m) * jax.lax.rsqrt(v + EPS) * g[:, None] + b[:, None]

    def dw3(x, wd, s):
        # depthwise 3x3x3, x (C, s, s, s), wd (C, 27)
        xp = jnp.pad(x, ((0, 0), (1, 1), (1, 1), (1, 1)))
        acc = None
        k = 0
        for a in range(3):
            for b_ in range(3):
                for c_ in range(3):
                    t = xp[:, a:a + s, b_:b_ + s, c_:c_ + s] * \
                        wd[:, k, None, None, None]
                    acc = t if acc is None else acc + t
                    k += 1
        return acc

    def up3(x):
        # (C, 8, 8, 8) -> (C, 16, 16, 16), align-corners linear, per axis
        y = jnp.einsum("Hh,chwd->cHwd", I16, x)
        y = jnp.einsum("Ww,cHwd->cHWd", I16, y)
        return jnp.einsum("Dd,cHWd->cHWD", I16, y)

    NX1 = IN_CH * 512

    def pre_glue(xal, w_ch, b_ch, gamma_l, beta_l, gamma_h, beta_h,
                 gamma2, beta2, kv_dw, kv_pw, q_dw, q_pw, out_dw, out_pw,
                 w_mlp):
        # xal (1, 81920): per-core shard of x1+x2 packed f16
        xa = jax.lax.all_gather(xal, "core", axis=0, tiled=True).reshape(-1)
        x1 = xa[:NX1].reshape(IN_CH, 512).astype(jnp.float32)      # (256,512)
        x2 = xa[NX1:].reshape(OUT_CH, N).astype(jnp.float32)       # (128,4096)

        residue = up3((w_ch @ x1 + b_ch[:, None]).reshape(OUT_CH, 8, 8, 8)
                      ).reshape(OUT_CH, N)                         # (128,4096)

        x1n = bn2(x1, gamma_l, beta_l)
        x2n = bn2(x2, gamma_h, beta_h)

        kv = kv_pw @ dw3(x1n.reshape(IN_CH, 8, 8, 8), kv_dw, 8
                         ).reshape(IN_CH, 512)                     # (256,512)
        k_ = up3(kv[:OUT_CH].reshape(OUT_CH, 8, 8, 8)).reshape(OUT_CH, N)
        v_ = up3(kv[OUT_CH:].reshape(OUT_CH, 8, 8, 8)).reshape(OUT_CH, N)
        q_ = q_pw @ dw3(x2n.reshape(OUT_CH, R, R, R), q_dw, R
                        ).reshape(OUT_CH, N)                       # (128,4096)

        # channel c = dd*HEADS + h ; kT[h, dd, j] = k_[dd*4+h, j]
        kT = k_.reshape(DIM_HEAD, HEADS, N).transpose(1, 0, 2).astype(bf)
        qT_all = q_.reshape(DIM_HEAD, HEADS, N).transpose(1, 0, 2).astype(bf)
        ci = jax.lax.axis_index("core")
        qT = jax.lax.dynamic_slice_in_dim(qT_all, ci * QPC, QPC, 2)

        # VA[p, (h*32+jc)*VPAD + col]: vh4[h,jc,p,dd] = v_[dd*4+h, jc*128+p]
        v4 = v_.reshape(DIM_HEAD, HEADS, 32, 128).transpose(1, 2, 3, 0)
        v4 = jnp.concatenate(
            [v4, jnp.ones((HEADS, 32, 128, 1), v4.dtype),
             jnp.zeros((HEADS, 32, 128, VPAD - 33), v4.dtype)], axis=3)
        va = v4.transpose(2, 0, 1, 3).reshape(128, HEADS * 32 * VPAD).astype(bf)

        zeros = jnp.zeros((HEADS, VPAD, QPC), jnp.float32)
        return kT, qT, va, zeros, residue

    pre = jax.jit(shard_map(
        pre_glue, mesh=mesh,
        in_specs=(P("core"),) + (P(),) * 15,
        out_specs=(P("core"), P("core"), P("core"), P("core"), P()),
        check_rep=False))

    def post_glue(otl, residue, gamma2, beta2, out_dw, out_pw, w_mlp):
        # otl (4, 36, 512) per-core; residue (128, 4096) replicated
        og = jax.lax.all_gather(otl, "core", axis=0, tiled=True)   # (32,36,512)
        og = og.reshape(NCORES, HEADS, VPAD, QPC)
        on = og[:, :, :32, :] / og[:, :, 32:33, :]                 # (8,4,32,512)
        # o_full[dd*4+h, c*512+i] = on[c,h,dd,i]
        o = on.transpose(2, 1, 0, 3).reshape(OUT_CH, N)
        o1 = out_pw @ dw3(o.reshape(OUT_CH, R, R, R), out_dw, R
                          ).reshape(OUT_CH, N)
        o1 = o1 + residue
        o2 = jnp.maximum(bn2(o1, gamma2, beta2), 0.0)
        return (w_mlp @ o2 + o1).reshape(1, OUT_CH, R, R, R).astype(
            jnp.float16)

    post = jax.jit(shard_map(
        post_glue, mesh=mesh,
        in_specs=(P("core"),) + (P(),) * 6,
        out_specs=P(),
        check_rep=False))

    ctx = {"jax": jax, "sharded": sharded, "pre": pre, "post": post,
           "shard": shard, "repl": repl}
    _CACHE["ctx"] = ctx
    return ctx


def _device_forward(x1, x2, W, rel_table):
    """x1 (1,256,8,8,8), x2 (1,128,16,16,16) f32; W dict of weights.
    Returns (1,128,16,16,16) f32."""
    from time import perf_counter as pc
    tt = _CACHE["t"] = {}
    t0 = pc()
    ctx = _get_ctx()
    jax = ctx["jax"]
    tt["ctx"] = pc() - t0

    # enqueue the activation upload first (single packed f16 buffer) so the
    # transfer overlaps with the host-side weight-key check below
    sh = ctx["shard"]
    xa = np.empty(IN_CH * 512 + OUT_CH * N, np.float16)
    xa[:IN_CH * 512] = x1.ravel()
    xa[IN_CH * 512:] = x2.ravel()
    xa_d = jax.device_put(xa.reshape(NCORES, -1), sh)
    tt["xput"] = pc() - t0

    # device-resident weights, keyed on bytes
    wkey = b"".join(W[n].tobytes() for n in WNAMES) + rel_table.tobytes()
    if _CACHE.get("wkey") != wkey:
        repl = ctx["repl"]
        wdev = []
        for n in WNAMES:
            w = W[n]
            if n.endswith("_dw"):
                w = w.reshape(w.shape[0], 27)
            elif w.ndim == 5:
                w = w.reshape(w.shape[0], w.shape[1])
            wdev.append(jax.device_put(np.ascontiguousarray(w), repl))
        _CACHE["wdev"] = wdev
        _CACHE["we_dev"] = jax.device_put(_build_we_concat(rel_table),
                                          ctx["shard"])
        _CACHE["wkey"] = wkey
    tt["wput"] = pc() - t0

    kT, qT, va, zeros, residue = ctx["pre"](xa_d, *_CACHE["wdev"])
    (ot,) = ctx["sharded"](kT, qT, va, _CACHE["we_dev"], zeros)
    wd = _CACHE["wdev"]
    # WNAMES order: ... gamma2(6) beta2(7) ... out_dw(12) out_pw(13) w_mlp(14)
    out = ctx["post"](ot, residue, wd[6], wd[7], wd[12], wd[13], wd[14])
    tt["dispatch"] = pc() - t0
    res = np.asarray(out)
    tt["fetch"] = pc() - t0
    return res


def _host_forward(x1, x2, W, rel_table):
    """Full-precision numpy fallback."""
    residue = _interp3(_pw(x1, W["w_ch"]) + W["b_ch"].reshape(1, -1, 1, 1, 1),
                       (R, R, R))
    x1n = _bn(x1, W["gamma_l"], W["beta_l"])
    x2n = _bn(x2, W["gamma_h"], W["beta_h"])
    kv = _pw(_dw(x1n, W["kv_dw"]), W["kv_pw"])
    k_, v_ = kv[:, :OUT_CH], kv[:, OUT_CH:]
    q_ = _pw(_dw(x2n, W["q_dw"]), W["q_pw"])
    k_ = _interp3(k_, (R, R, R))
    v_ = _interp3(v_, (R, R, R))

    def heads_split(t):
        b, c = t.shape[0], t.shape[1]
        t = t.reshape(b, DIM_HEAD, HEADS, -1)
        return np.ascontiguousarray(t[0].transpose(1, 2, 0))  # (4,4096,32)

    qh, kh, vh = heads_split(q_), heads_split(k_), heads_split(v_)

    u = _u_vec()
    te = _t_ext(rel_table)
    o = np.zeros((HEADS, N, 32), np.float32)
    m = u[:, None] - u[None, :] + 480 + 15                     # (4096,4096)
    for h in range(HEADS):
        bias = te[h][m]
        logits = (qh[h] @ kh[h].T + bias) * SCALE
        logits -= logits.max(axis=-1, keepdims=True)
        p = np.exp(logits)
        p /= p.sum(axis=-1, keepdims=True)
        o[h] = p @ vh[h]

    o_full = np.zeros((OUT_CH, N), np.float32)
    for h in range(HEADS):
        o_full[h::HEADS, :] = o[h].T
    o_sp = o_full.reshape(1, OUT_CH, R, R, R)

    o1 = _pw(_dw(o_sp, W["out_dw"]), W["out_pw"])
    o1 = o1 + residue
    res2 = o1
    o2 = np.maximum(_bn(o1, W["gamma2"], W["beta2"]), 0.0)
    o3 = _pw(o2, W["w_mlp"])
    return (o3 + res2).astype(np.float32)


# ---------------- main entry ----------------

def kernel(x1, x2, w_ch, b_ch, gamma_l, beta_l, gamma_h, beta_h, gamma2,
           beta2, kv_dw, kv_pw, q_dw, q_pw, out_dw, out_pw, w_mlp, rel_table):
    x1 = np.ascontiguousarray(np.asarray(x1, np.float32))
    x2 = np.ascontiguousarray(np.asarray(x2, np.float32))
    rel_table = np.ascontiguousarray(np.asarray(rel_table, np.float32))
    W = {n: np.ascontiguousarray(np.asarray(v, np.float32)) for n, v in [
        ("w_ch", w_ch), ("b_ch", b_ch), ("gamma_l", gamma_l),
        ("beta_l", beta_l), ("gamma_h", gamma_h), ("beta_h", beta_h),
        ("gamma2", gamma2), ("beta2", beta2), ("kv_dw", kv_dw),
        ("kv_pw", kv_pw), ("q_dw", q_dw), ("q_pw", q_pw),
        ("out_dw", out_dw), ("out_pw", out_pw), ("w_mlp", w_mlp)]}

    try:
        out = _device_forward(x1, x2, W, rel_table)
        if not np.isfinite(out).all():
            raise FloatingPointError("non-finite device output")
    except Exception as exc:  # insurance: keep output correct
        print(f"[kernel] device path failed ({exc!r}); numpy fallback",
              file=sys.stderr)
        out = _host_forward(x1, x2, W, rel_table)
    return np.asarray(out, np.float32)


# revision 8
# speedup vs baseline: 1.6259x; 1.6259x over previous
"""BasicTransDecoderBlock on 8 Trainium2 NeuronCores — full on-device pipeline.

Architecture (one warm call = 2 async H2D puts + 4 chained async execs +
1 blocking fetch, ~1 tunnel RTT total):

  1. x1 (256,512) and x2 (128,4096) ship f16, sharded over channels
     (1.25MB wire).
  2. pre-glue shard_map jit: BN + 27-tap depthwise conv run on the LOCAL
     channel shard (both are per-channel ops), the depthwise output is
     all-gathered, then pointwise matmuls + align-corners interp (einsums
     with a constant (16,8) matrix) + head-split layouts emit the
     per-core bass inputs kT/qT/VA (bf16) + donated OT zeros.
  3. residue shard_map jit (separate small module; fusing it into
     pre-glue triggers a ~40ms neuronx codegen pathology).
  4. bass attention exec (unchanged math from the working baseline):
     4 heads x 4096 queries x 4096 keys, query-sharded 512/core.
     exp(s(qk+bias)) = exp(s qk) * WE window table (rel-pos bias is
     affine in u=31h+w+d; each core's queries span 62 consecutive u),
     softmax normalization folded into AV via an appended ones column.
  5. post-glue shard_map jit: all-gather OT, normalize, reassemble
     channels, out conv + residual + BN/relu + mlp conv + residual,
     replicated f16 output.
  6. single np.asarray fetch of the replicated (1,128,16,16,16) result.

Weights (incl. the host-built 16MB WE exp-bias table, a pure function of
rel_table) are cached device-resident keyed on their bytes.
"""

import sys
import numpy as np

sys.path.insert(0, "/opt/trn_rl_repo")

import ml_dtypes

IN_CH, OUT_CH, HEADS, DIM_HEAD, R = 256, 128, 4, 32, 16
EPS = 1e-5
SCALE = DIM_HEAD ** -0.5
N = R * R * R          # 4096 keys / queries
QPC = N // 8           # 512 queries per core
CWIN = 62              # distinct u values per core (2 h-planes)
CPAD = 64              # padded bias window stride
VPAD = 36              # 32 dims + 1 ones col, padded
NCORES = 8

WNAMES = ["w_ch", "b_ch", "gamma_l", "beta_l", "gamma_h", "beta_h",
          "gamma2", "beta2", "kv_dw", "kv_pw", "q_dw", "q_pw",
          "out_dw", "out_pw", "w_mlp"]


# ---------------- host-side numpy reference pieces (fallback) ----------------

def _pw(x, w):
    b, c = x.shape[0], x.shape[1]
    xf = x.reshape(b, c, -1)
    o = np.einsum("oi,bif->bof", w.reshape(w.shape[0], c), xf)
    return o.reshape(b, w.shape[0], *x.shape[2:])


def _dw(x, wd):
    b, c, h, w, d = x.shape
    xp = np.zeros((b, c, h + 2, w + 2, d + 2), x.dtype)
    xp[:, :, 1:-1, 1:-1, 1:-1] = x
    out = np.zeros_like(x)
    for a in range(3):
        for bb in range(3):
            for cc in range(3):
                out += wd[None, :, 0, a, bb, cc, None, None, None] * \
                    xp[:, :, a:a + h, bb:bb + w, cc:cc + d]
    return out


def _bn(x, g, b):
    m = x.mean(axis=(0, 2, 3, 4), keepdims=True, dtype=np.float32)
    v = ((x - m) ** 2).mean(axis=(0, 2, 3, 4), keepdims=True, dtype=np.float32)
    return (x - m) / np.sqrt(v + EPS) * g.reshape(1, -1, 1, 1, 1) + \
        b.reshape(1, -1, 1, 1, 1)


def _interp_mat(out_len, in_len):
    I = np.zeros((out_len, in_len), np.float32)
    pos = np.arange(out_len, dtype=np.float64) * ((in_len - 1) / (out_len - 1))
    lo = np.clip(np.floor(pos).astype(np.int64), 0, in_len - 1)
    hi = np.clip(lo + 1, 0, in_len - 1)
    w = (pos - lo).astype(np.float32)
    for o in range(out_len):
        I[o, lo[o]] += 1.0 - w[o]
        I[o, hi[o]] += w[o]
    return I


def _interp1(x, axis, out_len):
    in_len = x.shape[axis]
    if in_len == out_len:
        return x
    I = _interp_mat(out_len, in_len)
    return np.moveaxis(np.tensordot(I, np.moveaxis(x, axis, 0), axes=1), 0, axis)


def _interp3(x, size):
    for ax, s in zip((2, 3, 4), size):
        x = _interp1(x, ax, s)
    return x


def _u_vec():
    hh, ww, dd = np.meshgrid(np.arange(R), np.arange(R), np.arange(R),
                             indexing="ij")
    return (31 * hh + ww + dd).reshape(-1)  # (4096,), u in [0,495]


def _t_ext(rel_table):
    # t_ext[h, m] for m in [-15, 975] stored at index m+15 -> length 991
    m = np.arange(-15, 976) % ((2 * R - 1) ** 3)
    return rel_table[m, :].T.astype(np.float32)  # (4, 991)


# ---------------- device kernel ----------------

_CACHE = {}


def _ap4(t, ap_dims):
    import concourse.bass as bass
    b = t if isinstance(t, bass.AP) else t[:]
    return bass.AP(tensor=b.tensor, offset=b.offset,
                   ap=[list(b.ap[0])] + ap_dims)


def _build_bass():
    import concourse.bass as bass
    import concourse.mybir as mybir
    from contextlib import ExitStack

    dt = mybir.dt
    nc = bass.Bass()
    kT = nc.dram_tensor("kT", [HEADS, 32, N], dt.bfloat16, kind="ExternalInput")
    qT = nc.dram_tensor("qT", [HEADS, 32, QPC], dt.bfloat16, kind="ExternalInput")
    VA = nc.dram_tensor("VA", [128, HEADS * 32 * VPAD], dt.bfloat16,
                        kind="ExternalInput")
    WE = nc.dram_tensor("WE", [128, HEADS * 32 * CPAD], dt.bfloat16,
                        kind="ExternalInput")
    OT = nc.dram_tensor("OT", [HEADS, VPAD, QPC], dt.float32,
                        kind="ExternalOutput")

    T = HEADS * 32  # 128 pipeline steps
    NB = 2          # double buffering

    with ExitStack() as ctx:
        en = ctx.enter_context
        kT_sb = en(nc.sbuf_tensor("kT_sb", [32, HEADS * N], dt.bfloat16))
        qT_sb = en(nc.sbuf_tensor("qT_sb", [32, HEADS * QPC], dt.bfloat16))
        va_sb = en(nc.sbuf_tensor("va_sb", [128, HEADS * 32 * VPAD], dt.bfloat16))
        we_sb = en(nc.sbuf_tensor("we_sb", [128, HEADS * 32 * CPAD], dt.bfloat16))
        e_sb = [en(nc.sbuf_tensor(f"e_sb{i}", [128, QPC], dt.bfloat16)) for i in range(NB)]
        p_sb = [en(nc.sbuf_tensor(f"p_sb{i}", [128, QPC], dt.bfloat16)) for i in range(NB)]
        ob_sb = [en(nc.sbuf_tensor(f"ob_sb{i}", [VPAD, QPC], dt.float32)) for i in range(HEADS)]
        pq_ps = [en(nc.psum_tensor(f"pq_ps{i}", [128, QPC], dt.float32)) for i in range(NB)]
        po_ps = [en(nc.psum_tensor(f"po_ps{i}", [VPAD, QPC], dt.float32)) for i in range(HEADS)]

        dmas = en(nc.semaphore("dmas"))
        qks = en(nc.semaphore("qks"))
        acts = en(nc.semaphore("acts"))
        dvs = en(nc.semaphore("dvs"))
        avs = en(nc.semaphore("avs"))
        cps = en(nc.semaphore("cps"))
        blk = en(nc.Block())

        NDMA = 2 * HEADS + 2

        @blk.sync
        def _(s):
            for h in range(HEADS):
                s.dma_start(kT_sb[:, h * N:(h + 1) * N], kT[h]).then_inc(dmas, 16)
                s.dma_start(qT_sb[:, h * QPC:(h + 1) * QPC], qT[h]).then_inc(dmas, 16)
            s.dma_start(va_sb[:], VA[:]).then_inc(dmas, 16)
            s.dma_start(we_sb[:], WE[:]).then_inc(dmas, 16)
            for h in range(HEADS):
                s.wait_ge(cps, h + 1)
                s.dma_start(OT[h], ob_sb[h][:]).then_inc(dmas, 16)

        @blk.tensor
        def _(t):
            t.wait_ge(dmas, 16 * NDMA)
            for ti in range(T):
                h, jc = ti // 32, ti % 32
                if ti >= 2:
                    t.wait_ge(acts, ti - 1)
                t.matmul(pq_ps[ti % NB][:],
                         kT_sb[:, h * N + jc * 128: h * N + (jc + 1) * 128],
                         qT_sb[:, h * QPC:(h + 1) * QPC],
                         start=True, stop=True).then_inc(qks, 1)
                if ti >= 1:
                    tp = ti - 1
                    hp, jp = tp // 32, tp % 32
                    t.wait_ge(dvs, tp + 1)
                    t.matmul(po_ps[hp][:],
                             va_sb[:, tp * VPAD:(tp + 1) * VPAD],
                             p_sb[tp % NB][:],
                             start=(jp == 0), stop=(jp == 31)).then_inc(avs, 1)
            tp = T - 1
            t.wait_ge(dvs, tp + 1)
            t.matmul(po_ps[HEADS - 1][:],
                     va_sb[:, tp * VPAD:(tp + 1) * VPAD],
                     p_sb[tp % NB][:],
                     start=False, stop=True).then_inc(avs, 1)

        @blk.scalar
        def _(s):
            for ti in range(T):
                s.wait_ge(qks, ti + 1)
                if ti >= NB:
                    s.wait_ge(dvs, ti - 1)
                s.activation(e_sb[ti % NB][:], pq_ps[ti % NB][:],
                             mybir.ActivationFunctionType.Exp,
                             scale=float(SCALE)).then_inc(acts, 1)

        @blk.vector
        def _(v):
            v.wait_ge(dmas, 16 * NDMA)
            for ti in range(T):
                v.wait_ge(acts, ti + 1)
                if ti >= NB:
                    v.wait_ge(avs, ti - 1)
                base = we_sb[:, ti * CPAD: ti * CPAD + CWIN]
                w_ap = _ap4(base, [[31, 2], [1, 16], [1, 16]])
                e4 = _ap4(e_sb[ti % NB], [[256, 2], [16, 16], [1, 16]])
                p4 = _ap4(p_sb[ti % NB], [[256, 2], [16, 16], [1, 16]])
                v.tensor_tensor(p4, e4, w_ap,
                                op=mybir.AluOpType.mult).then_inc(dvs, 1)
            for h in range(HEADS):
                v.wait_ge(avs, 32 * (h + 1))
                v.tensor_copy(ob_sb[h][:], po_ps[h][:]).then_inc(cps, 1)
    return nc


def _build_we_concat(rel_table):
    """Per-core exp-bias window tables, concatenated core-major:
    (8*128, HEADS*32*CPAD) bf16."""
    bf16 = ml_dtypes.bfloat16
    u = _u_vec()
    te = _t_ext(rel_table)                                     # (4, 991)
    tabs = np.exp(SCALE * te)                                  # (4, 991)
    cc = np.arange(CWIN)
    base_midx = cc[None, :] + 495 - u[:, None]                 # (4096, 62)
    out = np.zeros((NCORES, 128, HEADS, 32, CPAD), np.float32)
    for c in range(NCORES):
        # WE[p, (h*32+jc)*CPAD + cc] = exp(SCALE * t_ext[h, cc + 62c - u_j + 480])
        # with j = jc*128 + p
        g = tabs[:, base_midx + 62 * c]                        # (4, 4096, 62)
        out[c, :, :, :, :CWIN] = g.reshape(HEADS, 32, 128, CWIN).transpose(2, 0, 1, 3)
    return np.ascontiguousarray(
        out.reshape(NCORES * 128, HEADS * 32 * CPAD)).astype(bf16)


def _get_ctx():
    """Build-once runner state: bass jit, pre/post glue jits, shardings."""
    if "ctx" in _CACHE:
        return _CACHE["ctx"]

    import jax
    import jax.numpy as jnp
    from jax.experimental.shard_map import shard_map
    from jax.sharding import Mesh, NamedSharding, PartitionSpec as P
    import concourse.mybir as mybir
    from concourse import bass2jax

    bass2jax.install_neuronx_cc_hook()
    nc = _build_bass()

    partition_name = (nc.partition_id_tensor.name
                      if nc.partition_id_tensor else None)
    in_names, out_names, out_avals = [], [], []
    for alloc in nc.m.functions[0].allocations:
        if not isinstance(alloc, mybir.MemoryLocationSet):
            continue
        name = alloc.memorylocations[0].name
        if alloc.kind == "ExternalInput":
            if name != partition_name:
                in_names.append(name)
        elif alloc.kind == "ExternalOutput":
            out_avals.append(jax.core.ShapedArray(
                tuple(alloc.tensor_shape), mybir.dt.np(alloc.dtype)))
            out_names.append(name)
    n_params, n_outs = len(in_names), len(out_names)
    all_names = list(in_names) + list(out_names)
    if partition_name is not None:
        all_names.append(partition_name)

    def _body(*args):
        operands = list(args)
        if partition_name is not None:
            operands.append(bass2jax.partition_id_tensor())
        outs = bass2jax._bass_exec_p.bind(
            *operands,
            out_avals=tuple(out_avals),
            in_names=tuple(all_names),
            out_names=tuple(out_names),
            lowering_input_output_aliases=(),
            sim_require_finite=True,
            sim_require_nnan=True,
            nc=nc,
        )
        return tuple(outs)

    devices = jax.devices()[:NCORES]
    assert len(devices) == NCORES, f"need {NCORES} cores, got {len(devices)}"
    mesh = Mesh(np.asarray(devices), ("core",))
    shard = NamedSharding(mesh, P("core"))
    repl = NamedSharding(mesh, P())
    in_specs = (P("core"),) * (n_params + n_outs)
    out_specs = (P("core"),) * n_outs
    donate = tuple(range(n_params, n_params + n_outs))
    sharded = jax.jit(
        shard_map(_body, mesh=mesh, in_specs=in_specs, out_specs=out_specs,
                  check_rep=False),
        donate_argnums=donate,
        keep_unused=True,
    )

    I16 = jnp.asarray(_interp_mat(R, 8))                       # (16, 8)
    bf = jnp.bfloat16

    def bn2(x, g, b):
        # x (C, S): batch stats over S
        m = jnp.mean(x, axis=1, keepdims=True)
        v = jnp.mean((x - m) * (x - m), axis=1, keepdims=True)
        return (x - m) * jax.lax.rsqrt(v + EPS) * g[:, None] + b[:, None]

    def dw3(x, wd, s):
        # depthwise 3x3x3, x (C, s, s, s), wd (C, 27)
        xp = jnp.pad(x, ((0, 0), (1, 1), (1, 1), (1, 1)))
        acc = None
        k = 0
        for a in range(3):
            for b_ in range(3):
                for c_ in range(3):
                    t = xp[:, a:a + s, b_:b_ + s, c_:c_ + s] * \
                        wd[:, k, None, None, None]
                    acc = t if acc is None else acc + t
                    k += 1
        return acc

    def up3(x):
        # (C, 8, 8, 8) -> (C, 16, 16, 16), align-corners linear, per axis
        y = jnp.einsum("Hh,chwd->cHwd", I16, x)
        y = jnp.einsum("Ww,cHwd->cHWd", I16, y)
        return jnp.einsum("Dd,cHWd->cHWD", I16, y)

    def pre_glue(x1l, x2l, gamma_l, beta_l, gamma_h, beta_h, kv_dw, kv_pw,
                 q_dw, q_pw):
        # x1l (32,512), x2l (16,4096): per-core channel shards, f16 wire.
        # BatchNorm stats and the depthwise conv are per-channel, so both
        # run exactly on the local shard; only the 27-tap depthwise OUTPUT
        # is all-gathered for the pointwise matmuls.  (Computing the full
        # x1- and x2-branches redundantly per core in one module triggered
        # a ~40ms codegen pathology; this local formulation runs in ~10ms.)
        ci = jax.lax.axis_index("core")
        g1 = jax.lax.dynamic_slice_in_dim(gamma_l, ci * 32, 32, 0)
        b1 = jax.lax.dynamic_slice_in_dim(beta_l, ci * 32, 32, 0)
        w1 = jax.lax.dynamic_slice_in_dim(kv_dw, ci * 32, 32, 0)
        g2 = jax.lax.dynamic_slice_in_dim(gamma_h, ci * 16, 16, 0)
        b2 = jax.lax.dynamic_slice_in_dim(beta_h, ci * 16, 16, 0)
        w2 = jax.lax.dynamic_slice_in_dim(q_dw, ci * 16, 16, 0)
        d1 = dw3(bn2(x1l.astype(jnp.float32), g1, b1).reshape(32, 8, 8, 8),
                 w1, 8).reshape(32, 512)
        d2 = dw3(bn2(x2l.astype(jnp.float32), g2, b2).reshape(16, R, R, R),
                 w2, R).reshape(16, N)
        dkv = jax.lax.all_gather(d1, "core", axis=0, tiled=True)   # (256,512)
        dq = jax.lax.all_gather(d2, "core", axis=0, tiled=True)    # (128,4096)

        kv = kv_pw @ dkv                                           # (256,512)
        k_ = up3(kv[:OUT_CH].reshape(OUT_CH, 8, 8, 8)).reshape(OUT_CH, N)
        v_ = up3(kv[OUT_CH:].reshape(OUT_CH, 8, 8, 8)).reshape(OUT_CH, N)
        q_ = q_pw @ dq                                             # (128,4096)

        # channel c = dd*HEADS + h ; kT[h, dd, j] = k_[dd*4+h, j]
        kT = k_.reshape(DIM_HEAD, HEADS, N).transpose(1, 0, 2).astype(bf)
        qT_all = q_.reshape(DIM_HEAD, HEADS, N).transpose(1, 0, 2).astype(bf)
        qT = jax.lax.dynamic_slice_in_dim(qT_all, ci * QPC, QPC, 2)

        # VA[p, (h*32+jc)*VPAD + col]: vh4[h,jc,p,dd] = v_[dd*4+h, jc*128+p]
        v4 = v_.reshape(DIM_HEAD, HEADS, 32, 128).transpose(1, 2, 3, 0)
        v4 = jnp.concatenate(
            [v4, jnp.ones((HEADS, 32, 128, 1), v4.dtype),
             jnp.zeros((HEADS, 32, 128, VPAD - 33), v4.dtype)], axis=3)
        va = v4.transpose(2, 0, 1, 3).reshape(128, HEADS * 32 * VPAD).astype(bf)

        zeros = jnp.zeros((HEADS, VPAD, QPC), jnp.float32)
        return kT, qT, va, zeros

    pre = jax.jit(shard_map(
        pre_glue, mesh=mesh,
        in_specs=(P("core"), P("core")) + (P(),) * 8,
        out_specs=(P("core"),) * 4,
        check_rep=False))

    def res_glue(x1l, w_ch, b_ch):
        # residue path runs as its own small module (folding it into
        # pre_glue retriggers the slow-codegen pathology)
        x1 = jax.lax.all_gather(x1l, "core", axis=0, tiled=True).astype(
            jnp.float32)                                           # (256,512)
        return up3((w_ch @ x1 + b_ch[:, None]).reshape(OUT_CH, 8, 8, 8)
                   ).reshape(OUT_CH, N)

    preR = jax.jit(shard_map(
        res_glue, mesh=mesh,
        in_specs=(P("core"),) + (P(),) * 2,
        out_specs=P(),
        check_rep=False))

    def post_glue(otl, residue, gamma2, beta2, out_dw, out_pw, w_mlp):
        # otl (4, 36, 512) per-core; residue (128, 4096) replicated
        og = jax.lax.all_gather(otl, "core", axis=0, tiled=True)   # (32,36,512)
        og = og.reshape(NCORES, HEADS, VPAD, QPC)
        on = og[:, :, :32, :] / og[:, :, 32:33, :]                 # (8,4,32,512)
        # o_full[dd*4+h, c*512+i] = on[c,h,dd,i]
        o = on.transpose(2, 1, 0, 3).reshape(OUT_CH, N)
        o1 = out_pw @ dw3(o.reshape(OUT_CH, R, R, R), out_dw, R
                          ).reshape(OUT_CH, N)
        o1 = o1 + residue
        o2 = jnp.maximum(bn2(o1, gamma2, beta2), 0.0)
        return (w_mlp @ o2 + o1).reshape(1, OUT_CH, R, R, R).astype(
            jnp.float16)

    post = jax.jit(shard_map(
        post_glue, mesh=mesh,
        in_specs=(P("core"),) + (P(),) * 6,
        out_specs=P(),
        check_rep=False))

    ctx = {"jax": jax, "sharded": sharded, "pre": pre, "preR": preR,
           "post": post, "shard": shard, "repl": repl}
    _CACHE["ctx"] = ctx
    return ctx


def _device_forward(x1, x2, W, rel_table):
    """x1 (1,256,8,8,8), x2 (1,128,16,16,16) f32; W dict of weights.
    Returns (1,128,16,16,16) f32."""
    from time import perf_counter as pc
    tt = _CACHE["t"] = {}
    t0 = pc()
    ctx = _get_ctx()
    jax = ctx["jax"]
    tt["ctx"] = pc() - t0

    # enqueue the activation uploads first (f16 wire, channel-sharded) so
    # the transfer overlaps with the host-side weight-key check below
    sh = ctx["shard"]
    x1_d = jax.device_put(x1.reshape(IN_CH, 512).astype(np.float16), sh)
    x2_d = jax.device_put(x2.reshape(OUT_CH, N).astype(np.float16), sh)
    tt["xput"] = pc() - t0

    # device-resident weights, keyed on bytes
    wkey = b"".join(W[n].tobytes() for n in WNAMES) + rel_table.tobytes()
    if _CACHE.get("wkey") != wkey:
        repl = ctx["repl"]
        wdev = []
        for n in WNAMES:
            w = W[n]
            if n.endswith("_dw"):
                w = w.reshape(w.shape[0], 27)
            elif w.ndim == 5:
                w = w.reshape(w.shape[0], w.shape[1])
            wdev.append(jax.device_put(np.ascontiguousarray(w), repl))
        _CACHE["wdev"] = wdev
        _CACHE["we_dev"] = jax.device_put(_build_we_concat(rel_table),
                                          ctx["shard"])
        _CACHE["wkey"] = wkey
    tt["wput"] = pc() - t0

    wd = _CACHE["wdev"]
    # WNAMES order: w_ch(0) b_ch(1) gamma_l(2) beta_l(3) gamma_h(4)
    # beta_h(5) gamma2(6) beta2(7) kv_dw(8) kv_pw(9) q_dw(10) q_pw(11)
    # out_dw(12) out_pw(13) w_mlp(14)
    kT, qT, va, zeros = ctx["pre"](x1_d, x2_d, wd[2], wd[3], wd[4], wd[5],
                                   wd[8], wd[9], wd[10], wd[11])
    residue = ctx["preR"](x1_d, wd[0], wd[1])
    (ot,) = ctx["sharded"](kT, qT, va, _CACHE["we_dev"], zeros)
    out = ctx["post"](ot, residue, wd[6], wd[7], wd[12], wd[13], wd[14])
    tt["dispatch"] = pc() - t0
    res = np.asarray(out)
    tt["fetch"] = pc() - t0
    return res


def _host_forward(x1, x2, W, rel_table):
    """Full-precision numpy fallback."""
    residue = _interp3(_pw(x1, W["w_ch"]) + W["b_ch"].reshape(1, -1, 1, 1, 1),
                       (R, R, R))
    x1n = _bn(x1, W["gamma_l"], W["beta_l"])
    x2n = _bn(x2, W["gamma_h"], W["beta_h"])
    kv = _pw(_dw(x1n, W["kv_dw"]), W["kv_pw"])
    k_, v_ = kv[:, :OUT_CH], kv[:, OUT_CH:]
    q_ = _pw(_dw(x2n, W["q_dw"]), W["q_pw"])
    k_ = _interp3(k_, (R, R, R))
    v_ = _interp3(v_, (R, R, R))

    def heads_split(t):
        b, c = t.shape[0], t.shape[1]
        t = t.reshape(b, DIM_HEAD, HEADS, -1)
        return np.ascontiguousarray(t[0].transpose(1, 2, 0))  # (4,4096,32)

    qh, kh, vh = heads_split(q_), heads_split(k_), heads_split(v_)

    u = _u_vec()
    te = _t_ext(rel_table)
    o = np.zeros((HEADS, N, 32), np.float32)
    m = u[:, None] - u[None, :] + 480 + 15                     # (4096,4096)
    for h in range(HEADS):
        bias = te[h][m]
        logits = (qh[h] @ kh[h].T + bias) * SCALE
        logits -= logits.max(axis=-1, keepdims=True)
        p = np.exp(logits)
        p /= p.sum(axis=-1, keepdims=True)
        o[h] = p @ vh[h]

    o_full = np.zeros((OUT_CH, N), np.float32)
    for h in range(HEADS):
        o_full[h::HEADS, :] = o[h].T
    o_sp = o_full.reshape(1, OUT_CH, R, R, R)

    o1 = _pw(_dw(o_sp, W["out_dw"]), W["out_pw"])
    o1 = o1 + residue
    res2 = o1
    o2 = np.maximum(_bn(o1, W["gamma2"], W["beta2"]), 0.0)
    o3 = _pw(o2, W["w_mlp"])
    return (o3 + res2).astype(np.float32)


# ---------------- main entry ----------------

def kernel(x1, x2, w_ch, b_ch, gamma_l, beta_l, gamma_h, beta_h, gamma2,
           beta2, kv_dw, kv_pw, q_dw, q_pw, out_dw, out_pw, w_mlp, rel_table):
    x1 = np.ascontiguousarray(np.asarray(x1, np.float32))
    x2 = np.ascontiguousarray(np.asarray(x2, np.float32))
    rel_table = np.ascontiguousarray(np.asarray(rel_table, np.float32))
    W = {n: np.ascontiguousarray(np.asarray(v, np.float32)) for n, v in [
        ("w_ch", w_ch), ("b_ch", b_ch), ("gamma_l", gamma_l),
        ("beta_l", beta_l), ("gamma_h", gamma_h), ("beta_h", beta_h),
        ("gamma2", gamma2), ("beta2", beta2), ("kv_dw", kv_dw),
        ("kv_pw", kv_pw), ("q_dw", q_dw), ("q_pw", q_pw),
        ("out_dw", out_dw), ("out_pw", out_pw), ("w_mlp", w_mlp)]}

    try:
        out = _device_forward(x1, x2, W, rel_table)
        if not np.isfinite(out).all():
            raise FloatingPointError("non-finite device output")
    except Exception as exc:  # insurance: keep output correct
        print(f"[kernel] device path failed ({exc!r}); numpy fallback",
              file=sys.stderr)
        out = _host_forward(x1, x2, W, rel_table)
    return np.asarray(out, np.float32)


# revision 9
# speedup vs baseline: 1.6821x; 1.0345x over previous
"""BasicTransDecoderBlock on 8 Trainium2 NeuronCores — full on-device pipeline.

Architecture (one warm call = 2 async H2D puts + 4 chained async execs +
1 blocking fetch, ~1 tunnel RTT total):

  1. x1 (256,512) and x2 (128,4096) ship f16, sharded over channels
     (1.25MB wire).
  2. pre-glue shard_map jit: BN + 27-tap depthwise conv run on the LOCAL
     channel shard (both are per-channel ops), the depthwise output is
     all-gathered, then pointwise matmuls + align-corners interp (einsums
     with a constant (16,8) matrix) + head-split layouts emit the
     per-core bass inputs kT/qT/VA (bf16) + donated OT zeros.
  3. residue shard_map jit (separate small module; fusing it into
     pre-glue triggers a ~40ms neuronx codegen pathology).
  4. bass attention exec (unchanged math from the working baseline):
     4 heads x 4096 queries x 4096 keys, query-sharded 512/core.
     exp(s(qk+bias)) = exp(s qk) * WE window table (rel-pos bias is
     affine in u=31h+w+d; each core's queries span 62 consecutive u),
     softmax normalization folded into AV via an appended ones column.
  5. post-glue shard_map jit: all-gather OT, normalize, reassemble
     channels, out conv + residual + BN/relu + mlp conv + residual,
     replicated f16 output.
  6. single np.asarray fetch of the replicated (1,128,16,16,16) result.

Weights (incl. the host-built 16MB WE exp-bias table, a pure function of
rel_table) are cached device-resident keyed on their bytes.
"""

import sys
import numpy as np

sys.path.insert(0, "/opt/trn_rl_repo")

import ml_dtypes

IN_CH, OUT_CH, HEADS, DIM_HEAD, R = 256, 128, 4, 32, 16
EPS = 1e-5
SCALE = DIM_HEAD ** -0.5
N = R * R * R          # 4096 keys / queries
QPC = N // 8           # 512 queries per core
CWIN = 62              # distinct u values per core (2 h-planes)
CPAD = 64              # padded bias window stride
VPAD = 36              # 32 dims + 1 ones col, padded
NCORES = 8

WNAMES = ["w_ch", "b_ch", "gamma_l", "beta_l", "gamma_h", "beta_h",
          "gamma2", "beta2", "kv_dw", "kv_pw", "q_dw", "q_pw",
          "out_dw", "out_pw", "w_mlp"]


# ---------------- host-side numpy reference pieces (fallback) ----------------

def _pw(x, w):
    b, c = x.shape[0], x.shape[1]
    xf = x.reshape(b, c, -1)
    o = np.einsum("oi,bif->bof", w.reshape(w.shape[0], c), xf)
    return o.reshape(b, w.shape[0], *x.shape[2:])


def _dw(x, wd):
    b, c, h, w, d = x.shape
    xp = np.zeros((b, c, h + 2, w + 2, d + 2), x.dtype)
    xp[:, :, 1:-1, 1:-1, 1:-1] = x
    out = np.zeros_like(x)
    for a in range(3):
        for bb in range(3):
            for cc in range(3):
                out += wd[None, :, 0, a, bb, cc, None, None, None] * \
                    xp[:, :, a:a + h, bb:bb + w, cc:cc + d]
    return out


def _bn(x, g, b):
    m = x.mean(axis=(0, 2, 3, 4), keepdims=True, dtype=np.float32)
    v = ((x - m) ** 2).mean(axis=(0, 2, 3, 4), keepdims=True, dtype=np.float32)
    return (x - m) / np.sqrt(v + EPS) * g.reshape(1, -1, 1, 1, 1) + \
        b.reshape(1, -1, 1, 1, 1)


def _interp_mat(out_len, in_len):
    I = np.zeros((out_len, in_len), np.float32)
    pos = np.arange(out_len, dtype=np.float64) * ((in_len - 1) / (out_len - 1))
    lo = np.clip(np.floor(pos).astype(np.int64), 0, in_len - 1)
    hi = np.clip(lo + 1, 0, in_len - 1)
    w = (pos - lo).astype(np.float32)
    for o in range(out_len):
        I[o, lo[o]] += 1.0 - w[o]
        I[o, hi[o]] += w[o]
    return I


def _interp1(x, axis, out_len):
    in_len = x.shape[axis]
    if in_len == out_len:
        return x
    I = _interp_mat(out_len, in_len)
    return np.moveaxis(np.tensordot(I, np.moveaxis(x, axis, 0), axes=1), 0, axis)


def _interp3(x, size):
    for ax, s in zip((2, 3, 4), size):
        x = _interp1(x, ax, s)
    return x


def _u_vec():
    hh, ww, dd = np.meshgrid(np.arange(R), np.arange(R), np.arange(R),
                             indexing="ij")
    return (31 * hh + ww + dd).reshape(-1)  # (4096,), u in [0,495]


def _t_ext(rel_table):
    # t_ext[h, m] for m in [-15, 975] stored at index m+15 -> length 991
    m = np.arange(-15, 976) % ((2 * R - 1) ** 3)
    return rel_table[m, :].T.astype(np.float32)  # (4, 991)


# ---------------- device kernel ----------------

_CACHE = {}


def _ap4(t, ap_dims):
    import concourse.bass as bass
    b = t if isinstance(t, bass.AP) else t[:]
    return bass.AP(tensor=b.tensor, offset=b.offset,
                   ap=[list(b.ap[0])] + ap_dims)


def _build_bass():
    import concourse.bass as bass
    import concourse.mybir as mybir
    from contextlib import ExitStack

    dt = mybir.dt
    nc = bass.Bass()
    kT = nc.dram_tensor("kT", [HEADS, 32, N], dt.bfloat16, kind="ExternalInput")
    qT = nc.dram_tensor("qT", [HEADS, 32, QPC], dt.bfloat16, kind="ExternalInput")
    VA = nc.dram_tensor("VA", [128, HEADS * 32 * VPAD], dt.bfloat16,
                        kind="ExternalInput")
    WE = nc.dram_tensor("WE", [128, HEADS * 32 * CPAD], dt.bfloat16,
                        kind="ExternalInput")
    OT = nc.dram_tensor("OT", [HEADS, VPAD, QPC], dt.float32,
                        kind="ExternalOutput")

    T = HEADS * 32  # 128 pipeline steps
    NB = 2          # double buffering

    with ExitStack() as ctx:
        en = ctx.enter_context
        kT_sb = en(nc.sbuf_tensor("kT_sb", [32, HEADS * N], dt.bfloat16))
        qT_sb = en(nc.sbuf_tensor("qT_sb", [32, HEADS * QPC], dt.bfloat16))
        va_sb = en(nc.sbuf_tensor("va_sb", [128, HEADS * 32 * VPAD], dt.bfloat16))
        we_sb = en(nc.sbuf_tensor("we_sb", [128, HEADS * 32 * CPAD], dt.bfloat16))
        e_sb = [en(nc.sbuf_tensor(f"e_sb{i}", [128, QPC], dt.bfloat16)) for i in range(NB)]
        p_sb = [en(nc.sbuf_tensor(f"p_sb{i}", [128, QPC], dt.bfloat16)) for i in range(NB)]
        ob_sb = [en(nc.sbuf_tensor(f"ob_sb{i}", [VPAD, QPC], dt.float32)) for i in range(HEADS)]
        pq_ps = [en(nc.psum_tensor(f"pq_ps{i}", [128, QPC], dt.float32)) for i in range(NB)]
        po_ps = [en(nc.psum_tensor(f"po_ps{i}", [VPAD, QPC], dt.float32)) for i in range(HEADS)]

        dmas = en(nc.semaphore("dmas"))
        qks = en(nc.semaphore("qks"))
        acts = en(nc.semaphore("acts"))
        dvs = en(nc.semaphore("dvs"))
        avs = en(nc.semaphore("avs"))
        cps = en(nc.semaphore("cps"))
        blk = en(nc.Block())

        NDMA = 2 * HEADS + 2

        @blk.sync
        def _(s):
            for h in range(HEADS):
                s.dma_start(kT_sb[:, h * N:(h + 1) * N], kT[h]).then_inc(dmas, 16)
                s.dma_start(qT_sb[:, h * QPC:(h + 1) * QPC], qT[h]).then_inc(dmas, 16)
            s.dma_start(va_sb[:], VA[:]).then_inc(dmas, 16)
            s.dma_start(we_sb[:], WE[:]).then_inc(dmas, 16)
            for h in range(HEADS):
                s.wait_ge(cps, h + 1)
                s.dma_start(OT[h], ob_sb[h][:]).then_inc(dmas, 16)

        @blk.tensor
        def _(t):
            t.wait_ge(dmas, 16 * NDMA)
            for ti in range(T):
                h, jc = ti // 32, ti % 32
                if ti >= 2:
                    t.wait_ge(acts, ti - 1)
                t.matmul(pq_ps[ti % NB][:],
                         kT_sb[:, h * N + jc * 128: h * N + (jc + 1) * 128],
                         qT_sb[:, h * QPC:(h + 1) * QPC],
                         start=True, stop=True).then_inc(qks, 1)
                if ti >= 1:
                    tp = ti - 1
                    hp, jp = tp // 32, tp % 32
                    t.wait_ge(dvs, tp + 1)
                    t.matmul(po_ps[hp][:],
                             va_sb[:, tp * VPAD:(tp + 1) * VPAD],
                             p_sb[tp % NB][:],
                             start=(jp == 0), stop=(jp == 31)).then_inc(avs, 1)
            tp = T - 1
            t.wait_ge(dvs, tp + 1)
            t.matmul(po_ps[HEADS - 1][:],
                     va_sb[:, tp * VPAD:(tp + 1) * VPAD],
                     p_sb[tp % NB][:],
                     start=False, stop=True).then_inc(avs, 1)

        @blk.scalar
        def _(s):
            for ti in range(T):
                s.wait_ge(qks, ti + 1)
                if ti >= NB:
                    s.wait_ge(dvs, ti - 1)
                s.activation(e_sb[ti % NB][:], pq_ps[ti % NB][:],
                             mybir.ActivationFunctionType.Exp,
                             scale=float(SCALE)).then_inc(acts, 1)

        @blk.vector
        def _(v):
            v.wait_ge(dmas, 16 * NDMA)
            for ti in range(T):
                v.wait_ge(acts, ti + 1)
                if ti >= NB:
                    v.wait_ge(avs, ti - 1)
                base = we_sb[:, ti * CPAD: ti * CPAD + CWIN]
                w_ap = _ap4(base, [[31, 2], [1, 16], [1, 16]])
                e4 = _ap4(e_sb[ti % NB], [[256, 2], [16, 16], [1, 16]])
                p4 = _ap4(p_sb[ti % NB], [[256, 2], [16, 16], [1, 16]])
                v.tensor_tensor(p4, e4, w_ap,
                                op=mybir.AluOpType.mult).then_inc(dvs, 1)
            for h in range(HEADS):
                v.wait_ge(avs, 32 * (h + 1))
                v.tensor_copy(ob_sb[h][:], po_ps[h][:]).then_inc(cps, 1)
    return nc


def _build_we_concat(rel_table):
    """Per-core exp-bias window tables, concatenated core-major:
    (8*128, HEADS*32*CPAD) bf16."""
    bf16 = ml_dtypes.bfloat16
    u = _u_vec()
    te = _t_ext(rel_table)                                     # (4, 991)
    tabs = np.exp(SCALE * te)                                  # (4, 991)
    cc = np.arange(CWIN)
    base_midx = cc[None, :] + 495 - u[:, None]                 # (4096, 62)
    out = np.zeros((NCORES, 128, HEADS, 32, CPAD), np.float32)
    for c in range(NCORES):
        # WE[p, (h*32+jc)*CPAD + cc] = exp(SCALE * t_ext[h, cc + 62c - u_j + 480])
        # with j = jc*128 + p
        g = tabs[:, base_midx + 62 * c]                        # (4, 4096, 62)
        out[c, :, :, :, :CWIN] = g.reshape(HEADS, 32, 128, CWIN).transpose(2, 0, 1, 3)
    return np.ascontiguousarray(
        out.reshape(NCORES * 128, HEADS * 32 * CPAD)).astype(bf16)


def _get_ctx():
    """Build-once runner state: bass jit, pre/post glue jits, shardings."""
    if "ctx" in _CACHE:
        return _CACHE["ctx"]

    import jax
    import jax.numpy as jnp
    from jax.experimental.shard_map import shard_map
    from jax.sharding import Mesh, NamedSharding, PartitionSpec as P
    import concourse.mybir as mybir
    from concourse import bass2jax

    bass2jax.install_neuronx_cc_hook()
    nc = _build_bass()

    partition_name = (nc.partition_id_tensor.name
                      if nc.partition_id_tensor else None)
    in_names, out_names, out_avals = [], [], []
    for alloc in nc.m.functions[0].allocations:
        if not isinstance(alloc, mybir.MemoryLocationSet):
            continue
        name = alloc.memorylocations[0].name
        if alloc.kind == "ExternalInput":
            if name != partition_name:
                in_names.append(name)
        elif alloc.kind == "ExternalOutput":
            out_avals.append(jax.core.ShapedArray(
                tuple(alloc.tensor_shape), mybir.dt.np(alloc.dtype)))
            out_names.append(name)
    n_params, n_outs = len(in_names), len(out_names)
    all_names = list(in_names) + list(out_names)
    if partition_name is not None:
        all_names.append(partition_name)

    def _body(*args):
        operands = list(args)
        if partition_name is not None:
            operands.append(bass2jax.partition_id_tensor())
        outs = bass2jax._bass_exec_p.bind(
            *operands,
            out_avals=tuple(out_avals),
            in_names=tuple(all_names),
            out_names=tuple(out_names),
            lowering_input_output_aliases=(),
            sim_require_finite=True,
            sim_require_nnan=True,
            nc=nc,
        )
        return tuple(outs)

    devices = jax.devices()[:NCORES]
    assert len(devices) == NCORES, f"need {NCORES} cores, got {len(devices)}"
    mesh = Mesh(np.asarray(devices), ("core",))
    shard = NamedSharding(mesh, P("core"))
    repl = NamedSharding(mesh, P())
    in_specs = (P("core"),) * (n_params + n_outs)
    out_specs = (P("core"),) * n_outs
    donate = tuple(range(n_params, n_params + n_outs))
    sharded = jax.jit(
        shard_map(_body, mesh=mesh, in_specs=in_specs, out_specs=out_specs,
                  check_rep=False),
        donate_argnums=donate,
        keep_unused=True,
    )

    I16 = jnp.asarray(_interp_mat(R, 8))                       # (16, 8)
    bf = jnp.bfloat16

    def bn2(x, g, b):
        # x (C, S): batch stats over S
        m = jnp.mean(x, axis=1, keepdims=True)
        v = jnp.mean((x - m) * (x - m), axis=1, keepdims=True)
        return (x - m) * jax.lax.rsqrt(v + EPS) * g[:, None] + b[:, None]

    def dw3(x, wd, s):
        # depthwise 3x3x3, x (C, s, s, s), wd (C, 27)
        xp = jnp.pad(x, ((0, 0), (1, 1), (1, 1), (1, 1)))
        acc = None
        k = 0
        for a in range(3):
            for b_ in range(3):
                for c_ in range(3):
                    t = xp[:, a:a + s, b_:b_ + s, c_:c_ + s] * \
                        wd[:, k, None, None, None]
                    acc = t if acc is None else acc + t
                    k += 1
        return acc

    def up3(x):
        # (C, 8, 8, 8) -> (C, 16, 16, 16), align-corners linear, per axis
        y = jnp.einsum("Hh,chwd->cHwd", I16, x)
        y = jnp.einsum("Ww,cHwd->cHWd", I16, y)
        return jnp.einsum("Dd,cHWd->cHWD", I16, y)

    def pre_glue(x1l, x2l, gamma_l, beta_l, gamma_h, beta_h, kv_dw, kv_pw,
                 q_dw, q_pw):
        # x1l (32,512), x2l (16,4096): per-core channel shards, f16 wire.
        # BatchNorm stats and the depthwise conv are per-channel, so both
        # run exactly on the local shard; only the 27-tap depthwise OUTPUT
        # is all-gathered for the pointwise matmuls.  (Computing the full
        # x1- and x2-branches redundantly per core in one module triggered
        # a ~40ms codegen pathology; this local formulation runs in ~10ms.)
        ci = jax.lax.axis_index("core")
        g1 = jax.lax.dynamic_slice_in_dim(gamma_l, ci * 32, 32, 0)
        b1 = jax.lax.dynamic_slice_in_dim(beta_l, ci * 32, 32, 0)
        w1 = jax.lax.dynamic_slice_in_dim(kv_dw, ci * 32, 32, 0)
        g2 = jax.lax.dynamic_slice_in_dim(gamma_h, ci * 16, 16, 0)
        b2 = jax.lax.dynamic_slice_in_dim(beta_h, ci * 16, 16, 0)
        w2 = jax.lax.dynamic_slice_in_dim(q_dw, ci * 16, 16, 0)
        d1 = dw3(bn2(x1l.astype(jnp.float32), g1, b1).reshape(32, 8, 8, 8),
                 w1, 8).reshape(32, 512)
        d2 = dw3(bn2(x2l.astype(jnp.float32), g2, b2).reshape(16, R, R, R),
                 w2, R).reshape(16, N)
        # one packed all-gather (collectives carry a fixed per-op cost)
        dg = jax.lax.all_gather(
            jnp.concatenate([d1.reshape(1, -1), d2.reshape(1, -1)], axis=1),
            "core", axis=0, tiled=True)                # (8, 32*512+16*4096)
        dkv = dg[:, :32 * 512].reshape(IN_CH, 512)                 # (256,512)
        dq = dg[:, 32 * 512:].reshape(OUT_CH, N)                   # (128,4096)

        kv = kv_pw @ dkv                                           # (256,512)
        k_ = up3(kv[:OUT_CH].reshape(OUT_CH, 8, 8, 8)).reshape(OUT_CH, N)
        v_ = up3(kv[OUT_CH:].reshape(OUT_CH, 8, 8, 8)).reshape(OUT_CH, N)
        q_ = q_pw @ dq                                             # (128,4096)

        # channel c = dd*HEADS + h ; kT[h, dd, j] = k_[dd*4+h, j]
        kT = k_.reshape(DIM_HEAD, HEADS, N).transpose(1, 0, 2).astype(bf)
        qT_all = q_.reshape(DIM_HEAD, HEADS, N).transpose(1, 0, 2).astype(bf)
        qT = jax.lax.dynamic_slice_in_dim(qT_all, ci * QPC, QPC, 2)

        # VA[p, (h*32+jc)*VPAD + col]: vh4[h,jc,p,dd] = v_[dd*4+h, jc*128+p]
        v4 = v_.reshape(DIM_HEAD, HEADS, 32, 128).transpose(1, 2, 3, 0)
        v4 = jnp.concatenate(
            [v4, jnp.ones((HEADS, 32, 128, 1), v4.dtype),
             jnp.zeros((HEADS, 32, 128, VPAD - 33), v4.dtype)], axis=3)
        va = v4.transpose(2, 0, 1, 3).reshape(128, HEADS * 32 * VPAD).astype(bf)

        zeros = jnp.zeros((HEADS, VPAD, QPC), jnp.float32)
        return kT, qT, va, zeros

    pre = jax.jit(shard_map(
        pre_glue, mesh=mesh,
        in_specs=(P("core"), P("core")) + (P(),) * 8,
        out_specs=(P("core"),) * 4,
        check_rep=False))

    def res_glue(x1l, w_ch, b_ch):
        # residue path runs as its own small module (folding it into
        # pre_glue retriggers the slow-codegen pathology)
        x1 = jax.lax.all_gather(x1l, "core", axis=0, tiled=True).astype(
            jnp.float32)                                           # (256,512)
        return up3((w_ch @ x1 + b_ch[:, None]).reshape(OUT_CH, 8, 8, 8)
                   ).reshape(OUT_CH, N)

    preR = jax.jit(shard_map(
        res_glue, mesh=mesh,
        in_specs=(P("core"),) + (P(),) * 2,
        out_specs=P(),
        check_rep=False))

    def post_glue(otl, residue, gamma2, beta2, out_dw, out_pw, w_mlp):
        # otl (4, 36, 512) per-core; residue (128, 4096) replicated
        og = jax.lax.all_gather(otl, "core", axis=0, tiled=True)   # (32,36,512)
        og = og.reshape(NCORES, HEADS, VPAD, QPC)
        on = og[:, :, :32, :] / og[:, :, 32:33, :]                 # (8,4,32,512)
        # o_full[dd*4+h, c*512+i] = on[c,h,dd,i]
        o = on.transpose(2, 1, 0, 3).reshape(OUT_CH, N)
        o1 = out_pw @ dw3(o.reshape(OUT_CH, R, R, R), out_dw, R
                          ).reshape(OUT_CH, N)
        o1 = o1 + residue
        o2 = jnp.maximum(bn2(o1, gamma2, beta2), 0.0)
        return (w_mlp @ o2 + o1).reshape(1, OUT_CH, R, R, R).astype(
            jnp.float16)

    post = jax.jit(shard_map(
        post_glue, mesh=mesh,
        in_specs=(P("core"),) + (P(),) * 6,
        out_specs=P(),
        check_rep=False))

    ctx = {"jax": jax, "sharded": sharded, "pre": pre, "preR": preR,
           "post": post, "shard": shard, "repl": repl}
    _CACHE["ctx"] = ctx
    return ctx


def _device_forward(x1, x2, W, rel_table):
    """x1 (1,256,8,8,8), x2 (1,128,16,16,16) f32; W dict of weights.
    Returns (1,128,16,16,16) f32."""
    from time import perf_counter as pc
    tt = _CACHE["t"] = {}
    t0 = pc()
    ctx = _get_ctx()
    jax = ctx["jax"]
    tt["ctx"] = pc() - t0

    # enqueue the activation uploads first (f16 wire, channel-sharded) so
    # the transfer overlaps with the host-side weight-key check below
    sh = ctx["shard"]
    x1_d = jax.device_put(x1.reshape(IN_CH, 512).astype(np.float16), sh)
    x2_d = jax.device_put(x2.reshape(OUT_CH, N).astype(np.float16), sh)
    tt["xput"] = pc() - t0

    # device-resident weights, keyed on bytes
    wkey = b"".join(W[n].tobytes() for n in WNAMES) + rel_table.tobytes()
    if _CACHE.get("wkey") != wkey:
        repl = ctx["repl"]
        wdev = []
        for n in WNAMES:
            w = W[n]
            if n.endswith("_dw"):
                w = w.reshape(w.shape[0], 27)
            elif w.ndim == 5:
                w = w.reshape(w.shape[0], w.shape[1])
            wdev.append(jax.device_put(np.ascontiguousarray(w), repl))
        _CACHE["wdev"] = wdev
        _CACHE["we_dev"] = jax.device_put(_build_we_concat(rel_table),
                                          ctx["shard"])
        _CACHE["wkey"] = wkey
    tt["wput"] = pc() - t0

    wd = _CACHE["wdev"]
    # WNAMES order: w_ch(0) b_ch(1) gamma_l(2) beta_l(3) gamma_h(4)
    # beta_h(5) gamma2(6) beta2(7) kv_dw(8) kv_pw(9) q_dw(10) q_pw(11)
    # out_dw(12) out_pw(13) w_mlp(14)
    kT, qT, va, zeros = ctx["pre"](x1_d, x2_d, wd[2], wd[3], wd[4], wd[5],
                                   wd[8], wd[9], wd[10], wd[11])
    residue = ctx["preR"](x1_d, wd[0], wd[1])
    (ot,) = ctx["sharded"](kT, qT, va, _CACHE["we_dev"], zeros)
    out = ctx["post"](ot, residue, wd[6], wd[7], wd[12], wd[13], wd[14])
    tt["dispatch"] = pc() - t0
    res = np.asarray(out)
    tt["fetch"] = pc() - t0
    return res


def _host_forward(x1, x2, W, rel_table):
    """Full-precision numpy fallback."""
    residue = _interp3(_pw(x1, W["w_ch"]) + W["b_ch"].reshape(1, -1, 1, 1, 1),
                       (R, R, R))
    x1n = _bn(x1, W["gamma_l"], W["beta_l"])
    x2n = _bn(x2, W["gamma_h"], W["beta_h"])
    kv = _pw(_dw(x1n, W["kv_dw"]), W["kv_pw"])
    k_, v_ = kv[:, :OUT_CH], kv[:, OUT_CH:]
    q_ = _pw(_dw(x2n, W["q_dw"]), W["q_pw"])
    k_ = _interp3(k_, (R, R, R))
    v_ = _interp3(v_, (R, R, R))

    def heads_split(t):
        b, c = t.shape[0], t.shape[1]
        t = t.reshape(b, DIM_HEAD, HEADS, -1)
        return np.ascontiguousarray(t[0].transpose(1, 2, 0))  # (4,4096,32)

    qh, kh, vh = heads_split(q_), heads_split(k_), heads_split(v_)

    u = _u_vec()
    te = _t_ext(rel_table)
    o = np.zeros((HEADS, N, 32), np.float32)
    m = u[:, None] - u[None, :] + 480 + 15                     # (4096,4096)
    for h in range(HEADS):
        bias = te[h][m]
        logits = (qh[h] @ kh[h].T + bias) * SCALE
        logits -= logits.max(axis=-1, keepdims=True)
        p = np.exp(logits)
        p /= p.sum(axis=-1, keepdims=True)
        o[h] = p @ vh[h]

    o_full = np.zeros((OUT_CH, N), np.float32)
    for h in range(HEADS):
        o_full[h::HEADS, :] = o[h].T
    o_sp = o_full.reshape(1, OUT_CH, R, R, R)

    o1 = _pw(_dw(o_sp, W["out_dw"]), W["out_pw"])
    o1 = o1 + residue
    res2 = o1
    o2 = np.maximum(_bn(o1, W["gamma2"], W["beta2"]), 0.0)
    o3 = _pw(o2, W["w_mlp"])
    return (o3 + res2).astype(np.float32)


# ---------------- main entry ----------------

def kernel(x1, x2, w_ch, b_ch, gamma_l, beta_l, gamma_h, beta_h, gamma2,
           beta2, kv_dw, kv_pw, q_dw, q_pw, out_dw, out_pw, w_mlp, rel_table):
    x1 = np.ascontiguousarray(np.asarray(x1, np.float32))
    x2 = np.ascontiguousarray(np.asarray(x2, np.float32))
    rel_table = np.ascontiguousarray(np.asarray(rel_table, np.float32))
    W = {n: np.ascontiguousarray(np.asarray(v, np.float32)) for n, v in [
        ("w_ch", w_ch), ("b_ch", b_ch), ("gamma_l", gamma_l),
        ("beta_l", beta_l), ("gamma_h", gamma_h), ("beta_h", beta_h),
        ("gamma2", gamma2), ("beta2", beta2), ("kv_dw", kv_dw),
        ("kv_pw", kv_pw), ("q_dw", q_dw), ("q_pw", q_pw),
        ("out_dw", out_dw), ("out_pw", out_pw), ("w_mlp", w_mlp)]}

    try:
        out = _device_forward(x1, x2, W, rel_table)
        if not np.isfinite(out).all():
            raise FloatingPointError("non-finite device output")
    except Exception as exc:  # insurance: keep output correct
        print(f"[kernel] device path failed ({exc!r}); numpy fallback",
              file=sys.stderr)
        out = _host_forward(x1, x2, W, rel_table)
    return np.asarray(out, np.float32)
